# revision 54
# baseline (speedup 1.0000x reference)
"""Trainium2 Bass kernel for sparse_attention scoring + softmax.

Full computation: attn = softmax((enc @ W^T + b) @ hidden) for
enc = encoder_outputs[0] [32768, 1024].

Structure (per core, sequence-parallel over 8 cores, 4096 rows each):
- Host precomputes v = W^T @ hidden (the b.hidden constant cancels in
  softmax) and quantizes enc/v to fp16 (softmax rel err ~2.7e-3, vs the
  2e-2 gate; products and accumulation stay fp32 on device).  fp16 halves
  the HBM stream to 8.4MB/core.
- Device streams enc partition-major (seq = p*32 + j, so every DMA line
  is contiguous and the attn store is contiguous) and computes energy
  columns e[p,j] = enc_row . v with the work split so the DVE and ACT
  engines pipeline at ~26us each: ~14 columns run entirely on the DVE as
  fused affine_mul_reduce ops (~1.2us/col, fp32 accumulate), the rest as
  paired fp16 DVE multiplies (16-bit packed mode, ~0.55us/col) whose
  free-dim sum lands on the ACT engine (activation-Copy accum_out,
  ~1.35us/col).
- Tail uses the per-PARTITION max m[p] as the local softmax stabilizer
  (no cross-partition reduction needed): a[p,j] = exp(e[p,j] - m[p]),
  s[p] = sum_j a[p,j].  Outputs a [4096] and packed (m, s) [256].
- Host merges exactly (log-sum-exp): M = max m, S = sum s*exp(m-M),
  and scales each partition row by exp(m-M)/S while unsharding.

No collectives: the ncfw first-collective barrier + AllGather pipeline
costs ~30-40us in this environment (measured), an order of magnitude
more than the 8-byte-per-core exchange it would perform.
"""

import sys

sys.path.insert(0, "/opt/trn_rl_repo")

from contextlib import ExitStack

import numpy as np

import concourse.bacc as bacc
import concourse.mybir as mybir
import concourse.tile as tile
from concourse.bass_utils import run_bass_kernel_spmd

N_CORES = 8
SEQ = 32768
HID = 1024
SHARD = SEQ // N_CORES      # 4096
N_COL = SHARD // 128        # 32

K_MAX = 8
ENC_BUFS = 4
SCHEDULE = [1, 1, 2, 4, 8, 8, 4, 2, 1, 1]
assert sum(SCHEDULE) == N_COL


def build_body(nc, tc, enc, vb, out, ms_out):
    f32 = mybir.dt.float32
    mx = mybir.AluOpType.max

    ctx = ExitStack()
    cpool = ctx.enter_context(tc.tile_pool(name="cpool", bufs=1))
    iopool = ctx.enter_context(tc.tile_pool(name="iopool", bufs=ENC_BUFS))
    wpool = ctx.enter_context(tc.tile_pool(name="wpool", bufs=2))
    spool = ctx.enter_context(tc.tile_pool(name="spool", bufs=3))
    jpool = ctx.enter_context(tc.tile_pool(name="jpool", bufs=2))

    f16 = mybir.dt.float16

    # v pre-broadcast on host in fp16, stored twice along the free dim so
    # one DVE multiply can cover two energy columns; on the scalar HWDGE
    # ring (putting it at the head of the sync ring delays every enc tile
    # and starves the engines mid-loop — measured regression).
    v_sb = cpool.tile([128, 2 * HID], f16)
    nc.scalar.dma_start(out=v_sb[:, :], in_=vb[:, :])



    # Early throwaway exp so the ACT_TABLE_LOAD runs during the main loop,
    # not in front of the tail exp.
    warm = wpool.tile([1, 1], f32, tag="warm")
    nc.scalar.activation(
        out=warm[:, :], in_=v_sb[0:1, 0:1],
        func=mybir.ActivationFunctionType.Exp,
    )

    # --- main loop: e_sb[p, j] = energy of shard-local seq = p*N_COL + j ---
    e_sb = cpool.tile([128, N_COL], f32)
    enc_r = enc.rearrange("(p j) h -> p j h", p=128)

    j0 = 0
    for t, kt in enumerate(SCHEDULE):
        buf = iopool.tile([128, K_MAX * HID], f16, tag="enc")
        bufv = buf.rearrange("p (k h) -> p k h", k=K_MAX)
        nc.sync.dma_start(out=bufv[:, 0:kt, :], in_=enc_r[:, j0:j0 + kt, :])
        # Per-column dot products: every multiply runs as a paired fp16 DVE
        # tensor_mul (16-bit packed mode, ~0.55us/col; v_sb holds v twice),
        # and the free-dim sum-reduces are routed period-4 [ACT,ACT,DVE,DVE]
        # so the ACT engine (activation Copy + accum_out, ~1.35us/col) and
        # the DVE (tensor_scalar + accum_out — single-tensor op, eligible
        # for the fast packed modes) chew columns concurrently.
        k = 0
        while k < kt:
            kw = 2 if k + 1 < kt else 1
            scratch = spool.tile([128, 2 * HID], f16, tag="scratch")
            nc.vector.tensor_mul(
                scratch[:, 0:kw * HID],
                buf[:, k * HID:(k + kw) * HID],
                v_sb[:, 0:kw * HID],
            )
            for q in range(kw):
                j = j0 + k + q
                if j % 4 >= 2:
                    junk2 = jpool.tile([128, HID], f16, tag="junk2")
                    nc.vector.tensor_scalar(
                        out=junk2[:, :],
                        in0=scratch[:, q * HID:(q + 1) * HID],
                        scalar1=1.0,
                        scalar2=0.0,
                        op0=mybir.AluOpType.mult,
                        op1=mybir.AluOpType.add,
                        accum_out=e_sb[:, j:j + 1],
                    )
                else:
                    junk = jpool.tile([128, HID], f16, tag="junk")
                    nc.scalar.activation(
                        out=junk[:, :], in_=scratch[:, q * HID:(q + 1) * HID],
                        func=mybir.ActivationFunctionType.Copy,
                        accum_out=e_sb[:, j:j + 1],
                    )
            k += kw
        j0 += kt

    # --- tail: per-partition softmax pieces, no cross-partition reduction ---
    m1 = wpool.tile([128, 1], f32, tag="m1", bufs=1)
    nc.vector.tensor_reduce(
        out=m1[:, :], in_=e_sb[:, :], axis=mybir.AxisListType.X, op=mx,
    )
    nm1 = wpool.tile([128, 1], f32, tag="nm1", bufs=1)
    nc.vector.tensor_scalar_mul(nm1[:, :], m1[:, :], -1.0)

    a_loc = cpool.tile([128, N_COL], f32)
    ssum = wpool.tile([128, 1], f32, tag="ssum", bufs=1)
    nc.scalar.activation(
        out=a_loc[:, :], in_=e_sb[:, :],
        func=mybir.ActivationFunctionType.Exp,
        bias=nm1[:, :], scale=1.0,
        accum_out=ssum[:, :],
    )

    pk = wpool.tile([128, 2], f32, tag="pk", bufs=1)
    nc.vector.tensor_copy(pk[:, 0:1], m1[:, :])
    nc.vector.tensor_copy(pk[:, 1:2], ssum[:, :])

    nc.sync.dma_start(out=out.rearrange("(p j) -> p j", p=128),
                      in_=a_loc[:, :])
    nc.scalar.dma_start(out=ms_out.rearrange("(p k) -> p k", k=2),
                        in_=pk[:, :])

    ctx.close()


def build_nc(n_cores=N_CORES, debug=False):
    nc = bacc.Bacc(
        "TRN2",
        target_bir_lowering=False,
        debug=debug,
        num_devices=n_cores,
    )
    enc = nc.dram_tensor("enc", [SHARD, HID], mybir.dt.float16, kind="ExternalInput")
    vb = nc.dram_tensor("vb", [128, 2 * HID], mybir.dt.float16, kind="ExternalInput")
    out = nc.dram_tensor("attn_part", [SHARD], mybir.dt.float32,
                         kind="ExternalOutput")
    ms = nc.dram_tensor("ms", [2 * 128], mybir.dt.float32, kind="ExternalOutput")
    with tile.TileContext(nc) as tc:
        build_body(nc, tc, enc.ap(), vb.ap(), out.ap(), ms.ap())
    nc.compile()
    return nc


_NC_CACHE = {}


def _get_nc():
    if "nc" not in _NC_CACHE:
        _NC_CACHE["nc"] = build_nc()
    return _NC_CACHE["nc"]


def make_in_maps(hidden, encoder_outputs, attn_w, attn_b=None, n_cores=N_CORES,
                 shard=SHARD):
    hidden = np.asarray(hidden, dtype=np.float32)
    enc = np.asarray(encoder_outputs, dtype=np.float32)[0]
    w = np.asarray(attn_w, dtype=np.float32)
    v = (w.T @ hidden).astype(np.float32)
    # fp16 streaming: halves the HBM traffic of the enc stream and enables
    # the DVE 16-bit 2x packed mode for the multiply.  Softmax rel err vs
    # the f32 reference is ~2.7e-3 (quantization of enc, v, and products;
    # accumulation stays fp32 on device).
    v16 = v.astype(np.float16)
    v2 = np.concatenate([v16] * 2)
    vb = np.ascontiguousarray(np.broadcast_to(v2[None, :], (128, v2.shape[0])))
    return [
        {
            "enc": np.ascontiguousarray(
                enc[i * shard:(i + 1) * shard, :].astype(np.float16)),
            "vb": vb,
        }
        for i in range(n_cores)
    ]


def run(in_maps, trace=False, **kwargs):
    nc = _get_nc()
    return run_bass_kernel_spmd(
        nc, in_maps, core_ids=list(range(N_CORES)), trace=trace, **kwargs
    )


def kernel(**inputs):
    in_maps = make_in_maps(
        inputs["hidden"], inputs["encoder_outputs"], inputs["attn_w"],
        inputs.get("attn_b"),
    )
    res = run(in_maps)
    parts = [
        np.asarray(res.results[i]["attn_part"], dtype=np.float32).reshape(128, N_COL)
        for i in range(N_CORES)
    ]
    ms = [
        np.asarray(res.results[i]["ms"], dtype=np.float32).reshape(128, 2)
        for i in range(N_CORES)
    ]
    m = np.stack([x[:, 0] for x in ms]).astype(np.float64)   # [8, 128]
    s = np.stack([x[:, 1] for x in ms]).astype(np.float64)   # [8, 128]
    M = m.max()
    w = np.exp(m - M)                                        # [8, 128]
    S = float((s * w).sum())
    scale = (w / S).astype(np.float32)                       # [8, 128]
    attn = np.concatenate(
        [(parts[i] * scale[i][:, None]).reshape(-1) for i in range(N_CORES)]
    )
    return attn[None, None, :]


# revision 57
# speedup vs baseline: 1.2680x; 1.2680x over previous
"""Trainium2 Bass kernel for sparse_attention scoring + softmax.

Full computation: attn = softmax((enc @ W^T + b) @ hidden) for
enc = encoder_outputs[0] [32768, 1024].

Structure (per core, sequence-parallel over 8 cores, 4096 rows each):
- Host precomputes v = W^T @ hidden (the b.hidden constant cancels in
  softmax) and quantizes enc/v to fp16 (softmax rel err ~2.7e-3, vs the
  2e-2 gate; products and accumulation stay fp32 on device).  fp16 halves
  the HBM stream to 8.4MB/core.
- Device streams enc partition-major (seq = p*32 + j, so every DMA line
  is contiguous and the attn store is contiguous) and computes energy
  columns e[p,j] = enc_row . v with the work split so the DVE and ACT
  engines pipeline at ~26us each: ~14 columns run entirely on the DVE as
  fused affine_mul_reduce ops (~1.2us/col, fp32 accumulate), the rest as
  paired fp16 DVE multiplies (16-bit packed mode, ~0.55us/col) whose
  free-dim sum lands on the ACT engine (activation-Copy accum_out,
  ~1.35us/col).
- Tail uses the per-PARTITION max m[p] as the local softmax stabilizer
  (no cross-partition reduction needed): a[p,j] = exp(e[p,j] - m[p]),
  s[p] = sum_j a[p,j].  Outputs a [4096] and packed (m, s) [256].
- Host merges exactly (log-sum-exp): M = max m, S = sum s*exp(m-M),
  and scales each partition row by exp(m-M)/S while unsharding.

No collectives: the ncfw first-collective barrier + AllGather pipeline
costs ~30-40us in this environment (measured), an order of magnitude
more than the 8-byte-per-core exchange it would perform.
"""

import sys

sys.path.insert(0, "/opt/trn_rl_repo")

from contextlib import ExitStack

import numpy as np

import concourse.bacc as bacc
import concourse.mybir as mybir
import concourse.tile as tile
from concourse.bass_utils import run_bass_kernel_spmd

N_CORES = 8
SEQ = 32768
HID = 1024
SHARD = SEQ // N_CORES      # 4096
N_COL = SHARD // 128        # 32

K_MAX = 8
ENC_BUFS = 4
SCHEDULE = [1, 1, 2, 4, 8, 8, 4, 2, 1, 1]
assert sum(SCHEDULE) == N_COL


def build_body(nc, tc, enc, vb, out, ms_out):
    f32 = mybir.dt.float32
    mx = mybir.AluOpType.max

    ctx = ExitStack()
    cpool = ctx.enter_context(tc.tile_pool(name="cpool", bufs=1))
    iopool = ctx.enter_context(tc.tile_pool(name="iopool", bufs=ENC_BUFS))
    wpool = ctx.enter_context(tc.tile_pool(name="wpool", bufs=2))
    spool = ctx.enter_context(tc.tile_pool(name="spool", bufs=4))
    jpool = ctx.enter_context(tc.tile_pool(name="jpool", bufs=3))

    f16 = mybir.dt.float16

    # v pre-broadcast on host in fp16, stored twice along the free dim so
    # one DVE multiply can cover two energy columns; on the scalar HWDGE
    # ring (putting it at the head of the sync ring delays every enc tile
    # and starves the engines mid-loop — measured regression).  Loaded as
    # two halves: the first columns only read v_sb[:, 0:HID], so they can
    # start as soon as the first half lands.
    v_sb = cpool.tile([128, 2 * HID], f16)
    nc.scalar.dma_start(out=v_sb[:, 0:HID], in_=vb[:, 0:HID])
    nc.scalar.dma_start(out=v_sb[:, HID:2 * HID], in_=vb[:, HID:2 * HID])



    # Early throwaway exp so the ACT_TABLE_LOAD runs during the main loop,
    # not in front of the tail exp.
    warm = wpool.tile([1, 1], f32, tag="warm")
    nc.scalar.activation(
        out=warm[:, :], in_=v_sb[0:1, 0:1],
        func=mybir.ActivationFunctionType.Exp,
    )

    # --- main loop: e_sb[p, j] = energy of shard-local seq = p*N_COL + j ---
    e_sb = cpool.tile([128, N_COL], f32)
    enc_r = enc.rearrange("(p j) h -> p j h", p=128)

    j0 = 0
    for t, kt in enumerate(SCHEDULE):
        buf = iopool.tile([128, K_MAX * HID], f16, tag="enc")
        bufv = buf.rearrange("p (k h) -> p k h", k=K_MAX)
        nc.sync.dma_start(out=bufv[:, 0:kt, :], in_=enc_r[:, j0:j0 + kt, :])
        # Per-column dot products, split 3 ways so the DVE and ACT engines
        # pipeline: every 3rd column runs entirely on the DVE as a fused
        # affine_mul_reduce; the other columns run as an fp16 DVE multiply
        # (16-bit packed mode, ~0.68us/col, paired when adjacent to halve
        # launch overhead) whose free-dim sum lands on the otherwise-idle
        # ACT engine (activation Copy + accum_out, ~1.35us/col).
        k = 0
        while k < kt:
            j = j0 + k
            if j % 7 in (2, 5, 6) or j == 31:
                scratch = spool.tile([128, 2 * HID], f16, tag="scratch")
                nc.vector.affine_mul_reduce(
                    out=scratch[:, 0:HID],
                    accum_out=e_sb[:, j:j + 1],
                    in0=buf[:, k * HID:(k + 1) * HID],
                    in1=v_sb[:, 0:HID],
                    scale=1.0,
                    bias=0.0,
                )
                k += 1
                continue
            kw = 2 if (k + 1 < kt and (j + 1) % 7 not in (2, 5, 6)) else 1
            scratch = spool.tile([128, 2 * HID], f16, tag="scratch")
            nc.vector.tensor_mul(
                scratch[:, 0:kw * HID],
                buf[:, k * HID:(k + kw) * HID],
                v_sb[:, 0:kw * HID],
            )
            for q in range(kw):
                junk = jpool.tile([128, HID], f16, tag="junk")
                nc.scalar.activation(
                    out=junk[:, :], in_=scratch[:, q * HID:(q + 1) * HID],
                    func=mybir.ActivationFunctionType.Copy,
                    accum_out=e_sb[:, j + q:j + q + 1],
                )
            k += kw
        j0 += kt

    # --- tail: per-partition softmax pieces, no cross-partition reduction ---
    m1 = wpool.tile([128, 1], f32, tag="m1", bufs=1)
    nc.vector.tensor_reduce(
        out=m1[:, :], in_=e_sb[:, :], axis=mybir.AxisListType.X, op=mx,
    )
    nm1 = wpool.tile([128, 1], f32, tag="nm1", bufs=1)
    nc.vector.tensor_scalar_mul(nm1[:, :], m1[:, :], -1.0)

    a_loc = cpool.tile([128, N_COL], f32)
    ssum = wpool.tile([128, 1], f32, tag="ssum", bufs=1)
    nc.scalar.activation(
        out=a_loc[:, :], in_=e_sb[:, :],
        func=mybir.ActivationFunctionType.Exp,
        bias=nm1[:, :], scale=1.0,
        accum_out=ssum[:, :],
    )

    pk = wpool.tile([128, 2], f32, tag="pk", bufs=1)
    nc.vector.tensor_copy(pk[:, 0:1], m1[:, :])
    nc.vector.tensor_copy(pk[:, 1:2], ssum[:, :])

    nc.sync.dma_start(out=out.rearrange("(p j) -> p j", p=128),
                      in_=a_loc[:, :])
    nc.scalar.dma_start(out=ms_out.rearrange("(p k) -> p k", k=2),
                        in_=pk[:, :])

    ctx.close()


def build_nc(n_cores=N_CORES, debug=False):
    nc = bacc.Bacc(
        "TRN2",
        target_bir_lowering=False,
        debug=debug,
        num_devices=n_cores,
    )
    enc = nc.dram_tensor("enc", [SHARD, HID], mybir.dt.float16, kind="ExternalInput")
    vb = nc.dram_tensor("vb", [128, 2 * HID], mybir.dt.float16, kind="ExternalInput")
    out = nc.dram_tensor("attn_part", [SHARD], mybir.dt.float32,
                         kind="ExternalOutput")
    ms = nc.dram_tensor("ms", [2 * 128], mybir.dt.float32, kind="ExternalOutput")
    with tile.TileContext(nc) as tc:
        build_body(nc, tc, enc.ap(), vb.ap(), out.ap(), ms.ap())
    nc.compile()
    return nc


_NC_CACHE = {}


def _get_nc():
    if "nc" not in _NC_CACHE:
        _NC_CACHE["nc"] = build_nc()
    return _NC_CACHE["nc"]


def make_in_maps(hidden, encoder_outputs, attn_w, attn_b=None, n_cores=N_CORES,
                 shard=SHARD):
    hidden = np.asarray(hidden, dtype=np.float32)
    enc = np.asarray(encoder_outputs, dtype=np.float32)[0]
    w = np.asarray(attn_w, dtype=np.float32)
    v = (w.T @ hidden).astype(np.float32)
    # fp16 streaming: halves the HBM traffic of the enc stream and enables
    # the DVE 16-bit 2x packed mode for the multiply.  Softmax rel err vs
    # the f32 reference is ~2.7e-3 (quantization of enc, v, and products;
    # accumulation stays fp32 on device).
    v16 = v.astype(np.float16)
    v2 = np.concatenate([v16] * 2)
    vb = np.ascontiguousarray(np.broadcast_to(v2[None, :], (128, v2.shape[0])))
    return [
        {
            "enc": np.ascontiguousarray(
                enc[i * shard:(i + 1) * shard, :].astype(np.float16)),
            "vb": vb,
        }
        for i in range(n_cores)
    ]


def run(in_maps, trace=False, **kwargs):
    nc = _get_nc()
    return run_bass_kernel_spmd(
        nc, in_maps, core_ids=list(range(N_CORES)), trace=trace, **kwargs
    )


def kernel(**inputs):
    in_maps = make_in_maps(
        inputs["hidden"], inputs["encoder_outputs"], inputs["attn_w"],
        inputs.get("attn_b"),
    )
    res = run(in_maps)
    parts = [
        np.asarray(res.results[i]["attn_part"], dtype=np.float32).reshape(128, N_COL)
        for i in range(N_CORES)
    ]
    ms = [
        np.asarray(res.results[i]["ms"], dtype=np.float32).reshape(128, 2)
        for i in range(N_CORES)
    ]
    m = np.stack([x[:, 0] for x in ms]).astype(np.float64)   # [8, 128]
    s = np.stack([x[:, 1] for x in ms]).astype(np.float64)   # [8, 128]
    M = m.max()
    w = np.exp(m - M)                                        # [8, 128]
    S = float((s * w).sum())
    scale = (w / S).astype(np.float32)                       # [8, 128]
    attn = np.concatenate(
        [(parts[i] * scale[i][:, None]).reshape(-1) for i in range(N_CORES)]
    )
    return attn[None, None, :]
